# revision 39
# baseline (speedup 1.0000x reference)
"""Llama4 MoE (T=1024, H=1024, I=2048, SI=4096, E=8, K=1) on 8 trn2 NeuronCores.

Sharding (expert-parallel + shared-TP, host-side combine):
  - core c gets expert c's gate/up/down weights, a 512-wide slice of the
    shared expert, and the full hidden states + router weights (all bf16,
    pre-packed host-side into SBUF-tile layouts so every DMA is one
    contiguous >=512B descriptor per partition).
  - Device router: logits for ALL tokens via a 4-term bf16 residual
    decomposition ((xb+xr)@(wb+wr), fp32 PSUM accumulation) — error ~1e-6
    vs the fp32 reference, i.e. fp32-equivalent, so the top-1 argmax matches
    the reference.  Each core compacts its expert's tokens into C capacity
    slots with a permutation matmul that also applies the sigmoid routing
    weight on the input, runs the expert MLP at N=C, and computes its
    shared-expert shard for all tokens.  All MLP matmuls are bf16 with fp32
    accumulation.  C is chosen at call time from the actual router loads
    (max load + margin, rounded up), so capacity adapts to the inputs.
  - Outputs: outT[H, T] bf16 (shared-expert partial, transposed),
    re[h, slot] bf16 (routed rows at capacity slots), and slotm (the
    device's slot index + selection mask per token).  No on-device scatter:
    the host scatter-adds re into the summed output using the DEVICE's own
    routing decisions (slotm), so host/device can never disagree.  Host sums
    outT over cores (= the module's shared-TP all-reduce).
"""

import functools
import numpy as np

T, H, I, SI, E = 1024, 1024, 2048, 4096, 8
NCORES = 8
SIS = SI // NCORES   # 512 shared-intermediate shard
P = 128
HO = H // P          # 8
TT = T // P          # 8
IT = I // P          # 16
ST = SIS // P        # 4
NQ = 4               # token quarters for shared gate/up
QF = T // NQ         # 256
NIB = I // 256       # 8 expert-intermediate slabs of 256 cols

_LAST_C = [160]      # capacity of the most recently built program


def _build_nc(C):
    import concourse.mybir as mybir
    import concourse.tile as tile
    from concourse import bacc

    F32 = mybir.dt.float32
    BF16 = mybir.dt.bfloat16
    AF = mybir.ActivationFunctionType
    ALU = mybir.AluOpType
    AX = mybir.AxisListType

    nc = bacc.Bacc(trn_type="TRN2")

    # All inputs pre-packed host-side to the exact SBUF tile layout:
    # one contiguous descriptor per partition, >=512B each.
    xtb_d = nc.dram_tensor("xtb", [NQ, P, HO * QF], BF16, kind="ExternalInput")
    xtr_d = nc.dram_tensor("xtr", [NQ, P, HO * QF], BF16, kind="ExternalInput")
    xb_d = nc.dram_tensor("xb", [TT, P, H], BF16, kind="ExternalInput")
    rw2_d = nc.dram_tensor("rw2", [P, 2 * HO * E], BF16, kind="ExternalInput")
    esel_d = nc.dram_tensor("esel", [P, E], F32, kind="ExternalInput")
    iotac_d = nc.dram_tensor("iotac", [P, C], F32, kind="ExternalInput")
    ltri_d = nc.dram_tensor("ltri", [P, P], F32, kind="ExternalInput")
    sg_d = nc.dram_tensor("sgb", [ST, P, HO * P], BF16, kind="ExternalInput")
    su_d = nc.dram_tensor("sub", [ST, P, HO * P], BF16, kind="ExternalInput")
    sd_d = nc.dram_tensor("sdb", [2, P, ST * 512], BF16, kind="ExternalInput")
    eg_d = nc.dram_tensor("egb", [NIB, P, HO * 256], BF16, kind="ExternalInput")
    eu_d = nc.dram_tensor("eub", [NIB, P, HO * 256], BF16, kind="ExternalInput")
    ed_d = nc.dram_tensor("edb", [HO, P, IT * P], BF16, kind="ExternalInput")
    outT_d = nc.dram_tensor("outT", [H, T], BF16, kind="ExternalOutput")
    re_d = nc.dram_tensor("re", [P, HO * C], BF16, kind="ExternalOutput")
    slotm_d = nc.dram_tensor("slotm", [P, 2 * TT], F32, kind="ExternalOutput")

    with tile.TileContext(nc) as tc:
        with (
            tc.tile_pool(name="persist", bufs=1) as pp,
            tc.tile_pool(name="wgu", bufs=16) as wp,
            tc.tile_pool(name="wed", bufs=6) as edp,
            tc.tile_pool(name="wsd", bufs=2) as sdp,
            tc.tile_pool(name="actq", bufs=3) as sq,
            tc.tile_pool(name="outst", bufs=16) as op,
            tc.tile_pool(name="small", bufs=2) as sp,
            tc.tile_pool(name="ps_q", bufs=3, space="PSUM") as psq,
            tc.tile_pool(name="ps_x", bufs=5, space="PSUM") as psx_p,
        ):
            # ---- priority loads: x^T quarter 0 + shared gate/up slabs ----
            xtbq = []
            xtrq = []
            sg_sl = [None] * ST
            su_sl = [None] * ST
            # split first loads in ko-halves so the first matmuls start earlier
            HH = HO // 2
            sg_sl[0] = pp.tile([P, HO, P], BF16, tag="sg0", name="sg_sl")
            nc.sync.dma_start(sg_sl[0][:, :HH, :], sg_d[0, :, :HH * P])
            for q in range(1):
                t = pp.tile([P, HO, QF], BF16, tag=f"xtbq{q}", name="xtbq")
                nc.sync.dma_start(t[:, :HH, :], xtb_d[q, :, :HH * QF])
                nc.sync.dma_start(sg_sl[0][:, HH:, :], sg_d[0, :, HH * P:])
                nc.sync.dma_start(t[:, HH:, :], xtb_d[q, :, HH * QF:])
                su_sl[0] = pp.tile([P, HO, P], BF16, tag="su0", name="su_sl")
                nc.sync.dma_start(su_sl[0], su_d[0, :, :])
                xtbq.append(t)
            for si in range(1, ST):
                g = pp.tile([P, HO, P], BF16, tag=f"sg{si}", name="sg_sl")
                nc.sync.dma_start(g, sg_d[si, :, :])
                sg_sl[si] = g
                u = pp.tile([P, HO, P], BF16, tag=f"su{si}", name="su_sl")
                nc.sync.dma_start(u, su_d[si, :, :])
                su_sl[si] = u
            for q in range(1, NQ):
                t = pp.tile([P, HO, QF], BF16, tag=f"xtbq{q}", name="xtbq")
                nc.sync.dma_start(t, xtb_d[q, :, :])
                xtbq.append(t)
            # constants + x residual (needed by router/combine, ~30us in)
            rw_sb = pp.tile([P, 2, HO, E], BF16, tag="rw", name="rw_sb")
            nc.sync.dma_start(rw_sb, rw2_d[:, :])
            esel_sb = pp.tile([P, E], F32, tag="esel", name="esel_sb")
            nc.sync.dma_start(esel_sb, esel_d[:, :])
            iotac = pp.tile([P, C], F32, tag="iotac", name="iotac")
            nc.sync.dma_start(iotac, iotac_d[:, :])
            ltri = pp.tile([P, P], F32, tag="ltri", name="ltri")
            nc.sync.dma_start(ltri, ltri_d[:, :])
            for q in range(NQ):
                t = pp.tile([P, HO, QF], BF16, tag=f"xtrq{q}", name="xtrq")
                nc.sync.dma_start(t, xtr_d[q, :, :])
                xtrq.append(t)
            xb_t = []
            for tt in range(TT):
                t = pp.tile([P, H], BF16, tag=f"xb{tt}", name="xb_t")
                nc.sync.dma_start(t, xb_d[tt, :, :])
                xb_t.append(t)

            allones8 = pp.tile([TT, P], F32, tag="allones8", name="allones8")
            nc.vector.memset(allones8, 1.0)
            onescol = pp.tile([P, 1], F32, tag="onescol", name="onescol")
            nc.vector.memset(onescol, 1.0)


            # ---- shared expert gate/up: gsT[si_p, st, t] bf16 ----
            gsT = pp.tile([P, ST, T], BF16, tag="gsT", name="gsT")
            for q in range(NQ):
                qsl = slice(q * QF, (q + 1) * QF)
                for si in range(ST):
                    psg = psq.tile([P, QF], F32, tag="psq", name="psg")
                    for ko in range(HO):
                        nc.tensor.matmul(psg, sg_sl[si][:, ko, :],
                                         xtbq[q][:, ko, :],
                                         start=(ko == 0), stop=(ko == HO - 1))
                    psu = psq.tile([P, QF], F32, tag="psq", name="psu")
                    for ko in range(HO):
                        nc.tensor.matmul(psu, su_sl[si][:, ko, :],
                                         xtbq[q][:, ko, :],
                                         start=(ko == 0), stop=(ko == HO - 1))
                    s1 = sq.tile([P, QF], F32, tag="s1", name="s1")
                    nc.scalar.activation(s1, psg, AF.Silu)
                    nc.vector.tensor_tensor(gsT[:, si, qsl], s1, psu, ALU.mult)

            # ---- router logits: 4-term bf16 residual split, fp32-exact ----
            L_sb = pp.tile([P, TT, E], F32, tag="L", name="L_sb")
            for tt in range(TT):
                q, o = tt // 2, (tt % 2) * P
                psL = psq.tile([P, E], F32, tag="psq", name="psL")
                k = 0
                for xs in (xtbq, xtrq):
                    for wi in (0, 1):
                        for ko in range(HO):
                            nc.tensor.matmul(psL, xs[q][:, ko, o:o + P],
                                             rw_sb[:, wi, ko, :],
                                             start=(k == 0), stop=(k == 31))
                            k += 1
                nc.vector.tensor_copy(L_sb[:, tt, :], psL)

            # ---- top-1 combine: mask m and weight combw, both [t_p, tt] ----
            maxc = sp.tile([P, TT], F32, tag="maxc", name="maxc")
            nc.vector.reduce_max(maxc, L_sb, axis=AX.X)
            w_sb = sp.tile([P, TT], F32, tag="wsb", name="w_sb")
            nc.scalar.activation(w_sb, maxc, AF.Sigmoid)
            eq = sp.tile([P, TT, E], F32, tag="eq", name="eq")
            nc.vector.tensor_tensor(eq, L_sb,
                                    maxc[:, :, None].to_broadcast([P, TT, E]),
                                    ALU.is_equal)
            nc.vector.tensor_tensor(eq, eq,
                                    esel_sb[:, None, :].to_broadcast([P, TT, E]),
                                    ALU.mult)
            m_sb = sp.tile([P, TT], F32, tag="m", name="m_sb")
            nc.vector.reduce_sum(m_sb, eq, axis=AX.X)
            combw = sp.tile([P, TT], F32, tag="combw", name="combw")
            nc.vector.tensor_tensor(combw, m_sb, w_sb, ALU.mult)

            # ---- capacity slots: slot[t] = #selected tokens before t ----
            ps_cs = psq.tile([P, TT], F32, tag="psq", name="ps_cs")
            nc.tensor.matmul(ps_cs, ltri, m_sb, start=True, stop=True)
            ps_t = psq.tile([TT, 1], F32, tag="psq", name="ps_t")
            nc.tensor.matmul(ps_t, m_sb, onescol, start=True, stop=True)
            sumsT = sp.tile([TT, 1], F32, tag="sumsT", name="sumsT")
            nc.vector.tensor_copy(sumsT, ps_t)
            LS = sp.tile([TT, TT], F32, tag="LS", name="LS")
            nc.vector.tensor_tensor(LS, ltri[:TT, :TT],
                                    sumsT.to_broadcast([TT, TT]), ALU.mult)
            ps_off = psq.tile([P, TT], F32, tag="psq", name="ps_off")
            nc.tensor.matmul(ps_off, allones8, LS, start=True, stop=True)
            slot = sp.tile([P, TT], F32, tag="slot", name="slot")
            nc.vector.tensor_copy(slot, ps_cs)
            nc.vector.tensor_tensor(slot, slot, ps_off, ALU.add)

            # export the device's routing decisions for the host scatter
            nc.gpsimd.dma_start(slotm_d[:, :TT], slot)
            nc.gpsimd.dma_start(slotm_d[:, TT:], m_sb)

            # ---- gather permutation perm[t_p, tt, j] = combw * (slot==j) ----
            # (unselected tokens have combw == 0, so collisions are harmless)
            combw_b = sp.tile([P, TT], BF16, tag="combwb", name="combw_b")
            nc.vector.tensor_copy(combw_b, combw)
            permb = pp.tile([P, TT, C], BF16, tag="perm", name="permb")
            for tt in range(TT):
                nc.vector.tensor_tensor(
                    permb[:, tt, :],
                    slot[:, tt:tt + 1].to_broadcast([P, C]),
                    iotac, ALU.is_equal)
                nc.vector.tensor_tensor(
                    permb[:, tt, :], permb[:, tt, :],
                    combw_b[:, tt:tt + 1].to_broadcast([P, C]), ALU.mult)

            # ---- shared down + store outT (PE filler during combine/perm) ----
            for hb in range(2):
                sd_sl = sdp.tile([P, ST, 512], BF16, tag="sd", name="sd_sl")
                nc.sync.dma_start(sd_sl, sd_d[hb, :, :])
                for hj in range(4):
                    ho = hb * 4 + hj
                    for nh in range(2):
                        nsl = slice(nh * 512, (nh + 1) * 512)
                        psd2 = psx_p.tile([P, 512], F32, tag="psx", name="psd2")
                        for sk in range(ST):
                            nc.tensor.matmul(psd2,
                                             sd_sl[:, sk, hj * P:(hj + 1) * P],
                                             gsT[:, sk, nsl],
                                             start=(sk == 0), stop=(sk == ST - 1))
                        o_t = op.tile([P, 512], BF16, tag="ot", name="o_t")
                        if hb == 1 and hj >= 2:
                            nc.scalar.activation(o_t, psd2, AF.Copy)
                        else:
                            nc.vector.tensor_copy(o_t, psd2)
                        nc.gpsimd.dma_start(outT_d[ho * P:(ho + 1) * P, nsl], o_t)

            # ---- gather: xeT[h_p, ho, j] = sum_t x[t, h] * perm[t, j] ----
            xeT = pp.tile([P, HO, C], BF16, tag="xeT", name="xeT")
            for ho in range(HO):
                psx = psq.tile([P, C], F32, tag="psq", name="psx")
                for tt in range(TT):
                    nc.tensor.matmul(psx, xb_t[tt][:, ho * P:(ho + 1) * P],
                                     permb[:, tt, :],
                                     start=(tt == 0), stop=(tt == TT - 1))
                nc.vector.tensor_copy(xeT[:, ho, :], psx)

            # ---- routed expert gate/up at capacity C -> gTe[i_p, it, j] ----
            gTe = pp.tile([P, IT, C], BF16, tag="gTe", name="gTe")
            for ib in range(NIB):
                eg_sl = wp.tile([P, HO, 256], BF16, tag="w4k", name="eg_sl")
                nc.sync.dma_start(eg_sl, eg_d[ib, :, :])
                eu_sl = wp.tile([P, HO, 256], BF16, tag="w4k", name="eu_sl")
                nc.sync.dma_start(eu_sl, eu_d[ib, :, :])
                for a in range(2):
                    it = ib * 2 + a
                    psg = psx_p.tile([P, C], F32, tag="psx", name="psg2")
                    for ko in range(HO):
                        nc.tensor.matmul(psg, eg_sl[:, ko, a * P:(a + 1) * P],
                                         xeT[:, ko, :],
                                         start=(ko == 0), stop=(ko == HO - 1))
                    psu = psx_p.tile([P, C], F32, tag="psx", name="psu2")
                    for ko in range(HO):
                        nc.tensor.matmul(psu, eu_sl[:, ko, a * P:(a + 1) * P],
                                         xeT[:, ko, :],
                                         start=(ko == 0), stop=(ko == HO - 1))
                    s1r = sq.tile([P, C], F32, tag="s1r", name="s1r")
                    nc.scalar.activation(s1r, psg, AF.Silu)
                    nc.vector.tensor_tensor(gTe[:, it, :], s1r, psu, ALU.mult)

            # ---- routed down at capacity C -> re[h_p, ho, j], store ----
            re_sb = pp.tile([P, HO, C], BF16, tag="re", name="re_sb")
            for ho in range(HO):
                ed_sl = edp.tile([P, IT, P], BF16, tag="wed", name="ed_sl")
                nc.sync.dma_start(ed_sl, ed_d[ho, :, :])
                psdn = psx_p.tile([P, C], F32, tag="psx", name="psdn")
                for ik in range(IT):
                    nc.tensor.matmul(psdn, ed_sl[:, ik, :], gTe[:, ik, :],
                                     start=(ik == 0), stop=(ik == IT - 1))
                nc.vector.tensor_copy(re_sb[:, ho, :], psdn)
                if ho % 2 == 1:
                    nc.scalar.dma_start(re_d[:, (ho - 1) * C:(ho + 1) * C],
                                        re_sb[:, ho - 1:ho + 1, :])

    nc.compile()
    return nc


@functools.lru_cache(maxsize=4)
def _get_nc_for(C):
    return _build_nc(C)


def _get_nc(C=None):
    return _get_nc_for(C if C is not None else _LAST_C[0])


def _bf16(a):
    import ml_dtypes
    return np.ascontiguousarray(a).astype(ml_dtypes.bfloat16)


def _pick_capacity(x, rw):
    """Capacity from the actual (host-approximated) router loads + margin."""
    logits = x @ rw.T
    top = np.argmax(logits, axis=1)
    maxload = int(np.bincount(top, minlength=E).max())
    return max(144, maxload + 4)


def _make_in_maps(inputs, C):
    f = lambda v: np.asarray(v, dtype=np.float32)
    x = f(inputs["hidden_states"])
    rw = f(inputs["router_weight"])
    sg = f(inputs["shared_gate"])
    su = f(inputs["shared_up"])
    sd = f(inputs["shared_down"])
    eg = f(inputs["expert_gate"])
    eu = f(inputs["expert_up"])
    ed = f(inputs["expert_down"])

    xT = np.ascontiguousarray(x.T)                      # [H, T]
    xTb = _bf16(xT)
    xTr = _bf16(xT - xTb.astype(np.float32))
    # [H, T] -> [NQ, P, HO*QF]
    pack_xt = lambda a: np.ascontiguousarray(
        a.reshape(HO, P, NQ, QF).transpose(2, 1, 0, 3).reshape(NQ, P, HO * QF))
    xtb = pack_xt(xTb)
    xtr = pack_xt(xTr)
    xb = _bf16(x).reshape(TT, P, H)

    rwT = np.ascontiguousarray(rw.T)                    # [H, E]
    rwb = _bf16(rwT)
    rwr = _bf16(rwT - rwb.astype(np.float32))
    # [2, H, E] -> [P, 2*HO*E]
    rw2 = np.ascontiguousarray(
        np.stack([rwb, rwr]).reshape(2, HO, P, E)
        .transpose(2, 0, 1, 3).reshape(P, 2 * HO * E))

    iotac = np.tile(np.arange(C, dtype=np.float32), (P, 1))
    # ltri[t', t] = 1 iff t' < t (strict upper in row-major = lhsT layout)
    ltri = np.triu(np.ones((P, P), dtype=np.float32), 1)

    # [H, cols] -> [nslab, P, HO*w] (w cols per slab)
    def pack_h(a, w):
        ns = a.shape[1] // w
        return np.ascontiguousarray(
            a.reshape(HO, P, ns, w).transpose(2, 1, 0, 3).reshape(ns, P, HO * w))

    # [rows, H] -> [nslab, P, nr*w]: rows split into nr chunks of P,
    # cols into nslab chunks of w
    def pack_r(a, w):
        nr = a.shape[0] // P
        ns = a.shape[1] // w
        return np.ascontiguousarray(
            a.reshape(nr, P, ns, w).transpose(2, 1, 0, 3).reshape(ns, P, nr * w))

    in_maps = []
    for c in range(NCORES):
        esel = np.zeros((P, E), dtype=np.float32)
        esel[:, c] = 1.0
        in_maps.append({
            "xtb": xtb,
            "xtr": xtr,
            "xb": xb,
            "rw2": rw2,
            "esel": esel,
            "iotac": iotac,
            "ltri": ltri,
            "sgb": pack_h(_bf16(sg[:, c * SIS:(c + 1) * SIS]), P),
            "sub": pack_h(_bf16(su[:, c * SIS:(c + 1) * SIS]), P),
            "sdb": pack_r(_bf16(sd[c * SIS:(c + 1) * SIS, :]), 512),
            "egb": pack_h(_bf16(eg[c]), 256),
            "eub": pack_h(_bf16(eu[c]), 256),
            "edb": pack_r(_bf16(ed[c]), P),
        })
    return in_maps


def _run(inputs, trace=False):
    from concourse.bass_utils import run_bass_kernel_spmd
    x = np.asarray(inputs["hidden_states"], dtype=np.float32)
    rw = np.asarray(inputs["router_weight"], dtype=np.float32)
    C = _pick_capacity(x, rw)
    _LAST_C[0] = C
    nc = _get_nc(C)
    in_maps = _make_in_maps(inputs, C)
    res = run_bass_kernel_spmd(nc, in_maps, core_ids=list(range(NCORES)),
                               trace=trace)

    # host combine: sum shared partials (TP all-reduce) + scatter routed rows
    # using the DEVICE's own slot/mask decisions.
    acc = np.zeros((H, T), dtype=np.float64)
    for r in res.results:
        acc += np.asarray(r["outT"]).astype(np.float64)
    out = np.ascontiguousarray(acc.T)     # [T, H]
    for c in range(NCORES):
        r = res.results[c]
        re = np.asarray(r["re"]).astype(np.float64)
        re = re.reshape(P, HO, C).transpose(1, 0, 2).reshape(H, C)
        slotm = np.asarray(r["slotm"], dtype=np.float32)
        slot_flat = slotm[:, :TT].T.reshape(T)    # token t = tt*P + p
        m_flat = slotm[:, TT:].T.reshape(T)
        sel = (m_flat > 0.5) & (slot_flat > -0.5) & (slot_flat < C - 0.5)
        idx = np.rint(slot_flat[sel]).astype(np.int64)
        out[sel] += re[:, idx].T
    return out.astype(np.float32), res


def kernel(**inputs) -> np.ndarray:
    out, _ = _run(inputs, trace=False)
    return out
